# revision 14
# baseline (speedup 1.0000x reference)
"""EquiNN kernel for Trainium2 (Bass, raw), 8-core data parallel, fp16 I/O.

Computes out = l*X + g*rowsum(X) + b for X [4096, 8192] f32.

The rel-err gate is 2e-2 against max|out| (~43), an abs budget of ~0.87.
Casting X to fp16 on the host and streaming fp16 both ways costs ~5e-4
rel err (vs the f64 reference) and halves the per-core DMA bytes:
8.4 MB in + 8.4 MB out. l/g/b are baked as immediates.

Measured on this part (v1/v2 traces): ALL DMA queues (SWDGE + both HWDGE
rings) share one ~430 GB/s per-core ceiling (SBUF AXI fabric), so the
wall is first_packet(9.6us, SWDGE Q7 spin-up) + 16.8MB/430 + drain. A
single HWDGE-ring DMA runs on one SDMA engine (~134 GB/s); SWDGE spreads
one DMA across all 16. Compute (~22us/engine) hides fully under the
fabric window, so the whole kernel is: saturate SWDGE q0 start to end.

Per core (shard [512, 8192] = 4 tiles of [128, 8192]):
  - gpsimd: 8 half-tile loads, then per-tile stores gated on affine sems,
    then the drain. Single FIFO queue=loads phase then stores phase, no
    fabric bubbles (store descs queue behind load descs).
  - DVE: rowsum [0:rd) per tile (tensor_reduce, 1x - dst [P,1] never
    packs), then sb = g*rs_d + rs_a' in ONE scalar_tensor_tensor, then
    the affine x = l*x + sb as two half-tile tensor_scalars (fp16 4x).
    Same-engine RAW chains between small ops are guarded with self-sem
    waits: raw Bass DVE has no hazard interlock, and the per-partition
    scalar AP of tensor_scalar is prefetched at dispatch.
  - ACT: rowsum [rd:COLS) per tile via Identity(scale=g, bias=b/n) with
    accum_out, so the accumulator directly yields g*sum + b = rs_a'.
"""

import os
from dataclasses import dataclass

import numpy as np

import concourse.bass as bass
from concourse import mybir
from concourse.bass_utils import run_bass_kernel_spmd

N_CORES = 8
ROWS, COLS = 4096, 8192
SHARD = ROWS // N_CORES  # 512 rows per core
P = 128                  # SBUF partitions
N_TILES = SHARD // P     # 4
HALF = COLS // 2         # 4096

LAST_PROFILE = {}


@dataclass(frozen=True)
class Cfg:
    ra_hi: int = 5824  # ACT accums [0:4096) (h0-gated, early) + [4096:ra_hi);
                       # DVE reduces [ra_hi:COLS). Balanced ~6.3us/tile


DEFAULT_CFG = Cfg()


def _build(l: float, g: float, b: float, cfg: Cfg = DEFAULT_CFG) -> bass.Bass:
    nc = bass.Bass()
    f16 = mybir.dt.float16
    f32 = mybir.dt.float32
    assert HALF < cfg.ra_hi < COLS

    # pre-register the const AP the ACT bias needs (only 0.0/1.0 ship)
    bias_val = b / HALF
    bias_t = nc.alloc_sbuf_tensor("const-bias", [P, 1], f32)
    nc.gpsimd.memset(bias_t.ap(), bias_val)
    nc.const_aps.aps[(f32, bias_val)] = bias_t.ap()

    X = nc.declare_dram_parameter("X", [SHARD, COLS], f16, isOutput=False)
    out = nc.declare_dram_parameter("out", [SHARD, COLS], f16, isOutput=True)
    Xg = X.rearrange("(t p) c -> t p c", p=P)
    outg = out.rearrange("(t p) c -> t p c", p=P)

    import contextlib

    with contextlib.ExitStack() as ctx:
        xt = [
            ctx.enter_context(nc.sbuf_tensor(f"xt{t}", [P, COLS], f16))
            for t in range(N_TILES)
        ]
        dump = ctx.enter_context(nc.sbuf_tensor("dump", [P, HALF], f16))
        rs_d = [
            ctx.enter_context(nc.sbuf_tensor(f"rsd{t}", [P, 1], f32))
            for t in range(N_TILES)
        ]
        rs_a = [
            ctx.enter_context(nc.sbuf_tensor(f"rsa{t}", [P, 1], f32))
            for t in range(N_TILES)
        ]
        rs_b = [
            ctx.enter_context(nc.sbuf_tensor(f"rsb{t}", [P, 1], f32))
            for t in range(N_TILES)
        ]
        rs_v = [
            ctx.enter_context(nc.sbuf_tensor(f"rsv{t}", [P, 1], f32))
            for t in range(N_TILES)
        ]
        sb = [
            ctx.enter_context(nc.sbuf_tensor(f"sb{t}", [P, 1], f32))
            for t in range(N_TILES)
        ]
        rs_warm = ctx.enter_context(nc.sbuf_tensor("rs_warm", [P, 1], f32))
        load_sems = [
            [ctx.enter_context(nc.semaphore(f"ld{t}_{k}")) for k in (0, 1)]
            for t in range(N_TILES)
        ]
        act_rs_sems = [
            ctx.enter_context(nc.semaphore(f"act_rs{t}")) for t in range(N_TILES)
        ]
        act_rsb_sems = [
            ctx.enter_context(nc.semaphore(f"act_rsb{t}")) for t in range(N_TILES)
        ]
        affine_sems = [
            ctx.enter_context(nc.semaphore(f"aff{t}")) for t in range(N_TILES)
        ]
        psem = ctx.enter_context(nc.semaphore("dve_pipe"))
        store_sem = ctx.enter_context(nc.semaphore("store_sem"))
        block = ctx.enter_context(nc.Block())

        # ---- gpsimd: all loads, then all stores, then drain -------------
        def gpsimd_prog(eng):
            for t in range(N_TILES):
                for k in (0, 1):
                    eng.dma_start(
                        xt[t][:, k * HALF:(k + 1) * HALF],
                        Xg[t][:, k * HALF:(k + 1) * HALF],
                    ).then_inc(load_sems[t][k], 16)
            eng.wait_ge(store_sem, 16 * N_TILES)

        # ---- DVE: partial rowsum, sb, full affine -----------------------
        def dve_prog(vector):
            for t in range(N_TILES):
                vector.wait_ge(load_sems[t][1], 16)
                nc.vector.reduce_sum(
                    rs_d[t][:], xt[t][:, cfg.ra_hi:COLS], axis=mybir.AxisListType.X
                ).then_inc(psem, 1)
                vector.wait_ge(psem, 3 * t + 1)  # rs_d committed
                vector.wait_ge(act_rs_sems[t], 1)
                vector.wait_ge(act_rsb_sems[t], 1)
                # v = rs_a' + rs_b'  (= g*sum[0:ra_hi) + b, from ACT halves)
                nc.vector.tensor_add(
                    rs_v[t][:], rs_a[t][:], rs_b[t][:]
                ).then_inc(psem, 1)
                vector.wait_ge(psem, 3 * t + 2)
                # sb = g*rs_d + v
                nc.vector.scalar_tensor_tensor(
                    sb[t][:], rs_d[t][:], g, rs_v[t][:],
                    op0=mybir.AluOpType.mult, op1=mybir.AluOpType.add,
                ).then_inc(psem, 1)
                vector.wait_ge(psem, 3 * t + 3)  # sb committed before the
                # affine dispatch prefetches it
                for k in (0, 1):
                    # h1 residency guaranteed transitively via act_rs_sems
                    nc.vector.tensor_scalar(
                        xt[t][:, k * HALF:(k + 1) * HALF],
                        xt[t][:, k * HALF:(k + 1) * HALF],
                        l, sb[t][:],
                        op0=mybir.AluOpType.mult, op1=mybir.AluOpType.add,
                    ).then_inc(affine_sems[t], 1)

        # ---- ACT: accum-rowsum of [rd:COLS) -----------------------------
        def act_prog(scalar):
            # dummy 1-col pass: hoists ACT_TABLE_LOAD (~1.3us) off the
            # critical path while loads are still in flight
            nc.scalar.activation(
                dump[:, 0:1], dump[:, 0:1],
                mybir.ActivationFunctionType.Identity,
                bias=b / HALF, scale=g,
                accum_out=rs_warm[:],
            )
            for t in range(N_TILES):
                # ca_a: h0 region, starts as soon as h0 lands
                scalar.wait_ge(load_sems[t][0], 16)
                nc.scalar.activation(
                    dump[:], xt[t][:, 0:HALF],
                    mybir.ActivationFunctionType.Identity,
                    bias=b / HALF, scale=g,
                    accum_out=rs_a[t][:],
                ).then_inc(act_rs_sems[t], 1)
                # ca_b: [HALF:ra_hi), needs h1
                scalar.wait_ge(load_sems[t][1], 16)
                nc.scalar.activation(
                    dump[:, 0:cfg.ra_hi - HALF], xt[t][:, HALF:cfg.ra_hi],
                    mybir.ActivationFunctionType.Identity,
                    bias=0.0, scale=g,
                    accum_out=rs_b[t][:],
                ).then_inc(act_rsb_sems[t], 1)
                if t == 2:
                    # t1's store, dispatched once aff(1) is surely done
                    scalar.wait_ge(affine_sems[1], 2)
                    scalar.dma_start(outg[1], xt[1][:]).then_inc(store_sem, 16)
                if t == 3:
                    scalar.wait_ge(affine_sems[3], 2)
                    scalar.dma_start(outg[3], xt[3][:]).then_inc(store_sem, 16)

        def sp_prog(eng):
            for t in (0, 2):
                eng.wait_ge(affine_sems[t], 2)
                eng.dma_start(outg[t], xt[t][:]).then_inc(store_sem, 16)

        block.gpsimd(gpsimd_prog)
        block.vector(dve_prog)
        block.scalar(act_prog)
        block.sync(sp_prog)

    return nc


def kernel(X: np.ndarray, l: np.ndarray, g: np.ndarray, b: np.ndarray) -> np.ndarray:
    cfg = DEFAULT_CFG
    nc = _build(float(l[0]), float(g[0]), float(b[0]), cfg)

    X16 = np.ascontiguousarray(X).astype(np.float16)
    shards = X16.reshape(N_CORES, SHARD, COLS)
    in_maps = [{"X": shards[i]} for i in range(N_CORES)]

    trace = os.environ.get("BASS_KERNEL_TRACE") == "1"
    res = run_bass_kernel_spmd(nc, in_maps, list(range(N_CORES)), trace=trace)
    if trace:
        LAST_PROFILE.update(
            exec_time_ns=res.exec_time_ns,
            mean_exec_time_ns=res.mean_exec_time_ns,
            trace=res.instructions_and_trace[1] if res.instructions_and_trace else None,
            profile_json=res.profile_json,
        )
    out16 = np.concatenate([res.results[i]["out"] for i in range(N_CORES)], axis=0)
    return out16.astype(np.float32)


# revision 15
# speedup vs baseline: 1.1122x; 1.1122x over previous
"""EquiNN kernel for Trainium2 (Bass, raw), 8-core data parallel, fp16 I/O.

Computes out = l*X + g*rowsum(X) + b for X [4096, 8192] f32.

The rel-err gate is 2e-2 against max|out| (~43), an abs budget of ~0.87.
Casting X to fp16 on the host and streaming fp16 both ways costs ~5e-4
rel err (vs the f64 reference) and halves the per-core DMA bytes:
8.4 MB in + 8.4 MB out. l/g/b are baked as immediates.

Measured on this part: per-core DMA tops out around 430 GB/s on SWDGE
(16 SDMA engines per DMA) and the 8 cores contend for chip HBM, so the
kernel is DMA-bound end to end; compute (~24us across DVE+ACT) hides
under the transfer window. HWDGE-ring stores measured slower in every
layout tried, so all traffic runs on the single SWDGE queue: its FIFO
naturally phases loads before stores with no fabric bubbles.

Per core (shard [512, 8192] = 4 tiles of [128, 8192]):
  - gpsimd: 8 half-tile loads, then per-tile stores gated on affine
    sems, then the drain.
  - DVE: rowsum [0:rd) per tile (tensor_reduce, 1x - a [P,1] dst never
    packs), then sb = g*rs_d + rs_a' in ONE scalar_tensor_tensor, then
    the affine x = l*x + sb as two half-tile tensor_scalars (fp16 4x).
    Same-engine RAW chains between small ops are guarded with self-sem
    waits: raw Bass DVE has no hazard interlock, and the per-partition
    scalar AP of tensor_scalar is prefetched at dispatch.
  - ACT: rowsum [rd:COLS) per tile via Identity(scale=g, bias=b/n) with
    accum_out, so the accumulator directly yields g*sum + b = rs_a'.
"""

import os
from dataclasses import dataclass

import numpy as np

import concourse.bass as bass
from concourse import mybir
from concourse.bass_utils import run_bass_kernel_spmd

N_CORES = 8
ROWS, COLS = 4096, 8192
SHARD = ROWS // N_CORES  # 512 rows per core
P = 128                  # SBUF partitions
N_TILES = SHARD // P     # 4
HALF = COLS // 2         # 4096

LAST_PROFILE = {}


@dataclass(frozen=True)
class Cfg:
    rd: int = 2304  # DVE reduces [0:rd) (1x, ~1.1ns/col); ACT accums the
                    # rest (~0.9-1.06ns/col); both land ~6us/tile


DEFAULT_CFG = Cfg()


def _build(l: float, g: float, b: float, cfg: Cfg = DEFAULT_CFG) -> bass.Bass:
    nc = bass.Bass()
    f16 = mybir.dt.float16
    f32 = mybir.dt.float32
    assert 0 < cfg.rd <= HALF

    # pre-register the const AP the ACT bias needs (only 0.0/1.0 ship)
    bias_val = b / (COLS - cfg.rd)
    bias_t = nc.alloc_sbuf_tensor("const-bias", [P, 1], f32)
    nc.gpsimd.memset(bias_t.ap(), bias_val)
    nc.const_aps.aps[(f32, bias_val)] = bias_t.ap()
    nc.all_engine_barrier()

    X = nc.declare_dram_parameter("X", [SHARD, COLS], f16, isOutput=False)
    out = nc.declare_dram_parameter("out", [SHARD, COLS], f16, isOutput=True)
    Xg = X.rearrange("(t p) c -> t p c", p=P)
    outg = out.rearrange("(t p) c -> t p c", p=P)

    import contextlib

    with contextlib.ExitStack() as ctx:
        xt = [
            ctx.enter_context(nc.sbuf_tensor(f"xt{t}", [P, COLS], f16))
            for t in range(N_TILES)
        ]
        dump = ctx.enter_context(nc.sbuf_tensor("dump", [P, COLS - cfg.rd], f16))
        rs_d = [
            ctx.enter_context(nc.sbuf_tensor(f"rsd{t}", [P, 1], f32))
            for t in range(N_TILES)
        ]
        rs_a = [
            ctx.enter_context(nc.sbuf_tensor(f"rsa{t}", [P, 1], f32))
            for t in range(N_TILES)
        ]
        sb = [
            ctx.enter_context(nc.sbuf_tensor(f"sb{t}", [P, 1], f32))
            for t in range(N_TILES)
        ]
        load_sems = [
            [ctx.enter_context(nc.semaphore(f"ld{t}_{k}")) for k in (0, 1)]
            for t in range(N_TILES)
        ]
        act_rs_sems = [
            ctx.enter_context(nc.semaphore(f"act_rs{t}")) for t in range(N_TILES)
        ]
        affine_sems = [
            ctx.enter_context(nc.semaphore(f"aff{t}")) for t in range(N_TILES)
        ]
        psem = ctx.enter_context(nc.semaphore("dve_pipe"))
        store_sem = ctx.enter_context(nc.semaphore("store_sem"))
        block = ctx.enter_context(nc.Block())

        # ---- gpsimd: all loads, then all stores, then drain -------------
        def gpsimd_prog(eng):
            for t in range(N_TILES):
                for k in (0, 1):
                    eng.dma_start(
                        xt[t][:, k * HALF:(k + 1) * HALF],
                        Xg[t][:, k * HALF:(k + 1) * HALF],
                    ).then_inc(load_sems[t][k], 16)
            for t in range(N_TILES):
                eng.wait_ge(affine_sems[t], 2)
                for k in (0, 1):
                    eng.dma_start(
                        outg[t][:, k * HALF:(k + 1) * HALF],
                        xt[t][:, k * HALF:(k + 1) * HALF],
                    ).then_inc(store_sem, 16)
            eng.wait_ge(store_sem, 16 * 2 * N_TILES)

        # ---- DVE: partial rowsum, sb, full affine -----------------------
        def dve_prog(vector):
            for t in range(N_TILES):
                vector.wait_ge(load_sems[t][0], 16)
                nc.vector.reduce_sum(
                    rs_d[t][:], xt[t][:, 0:cfg.rd], axis=mybir.AxisListType.X
                ).then_inc(psem, 1)
                vector.wait_ge(psem, 2 * t + 1)  # rs_d committed
                vector.wait_ge(act_rs_sems[t], 1)
                # sb = g*rs_d + rs_a'   (rs_a' = g*act_sum + b, from ACT)
                nc.vector.scalar_tensor_tensor(
                    sb[t][:], rs_d[t][:], g, rs_a[t][:],
                    op0=mybir.AluOpType.mult, op1=mybir.AluOpType.add,
                ).then_inc(psem, 1)
                vector.wait_ge(psem, 2 * t + 2)  # sb committed before the
                # affine dispatch prefetches it
                for k in (0, 1):
                    # h1 residency guaranteed transitively via act_rs_sems
                    nc.vector.tensor_scalar(
                        xt[t][:, k * HALF:(k + 1) * HALF],
                        xt[t][:, k * HALF:(k + 1) * HALF],
                        l, sb[t][:],
                        op0=mybir.AluOpType.mult, op1=mybir.AluOpType.add,
                    ).then_inc(affine_sems[t], 1)

        # ---- ACT: accum-rowsum of [rd:COLS) -----------------------------
        def act_prog(scalar):
            n_act = COLS - cfg.rd
            for t in range(N_TILES):
                scalar.wait_ge(load_sems[t][1], 16)
                if cfg.rd < HALF:
                    scalar.wait_ge(load_sems[t][0], 16)
                # accum = sum(g*x + b/n) = g*sum(x) + b
                nc.scalar.activation(
                    dump[:], xt[t][:, cfg.rd:COLS],
                    mybir.ActivationFunctionType.Identity,
                    bias=b / n_act, scale=g,
                    accum_out=rs_a[t][:],
                ).then_inc(act_rs_sems[t], 1)

        block.gpsimd(gpsimd_prog)
        block.vector(dve_prog)
        block.scalar(act_prog)

    return nc


def kernel(X: np.ndarray, l: np.ndarray, g: np.ndarray, b: np.ndarray) -> np.ndarray:
    cfg = DEFAULT_CFG
    nc = _build(float(l[0]), float(g[0]), float(b[0]), cfg)

    X16 = np.ascontiguousarray(X).astype(np.float16)
    shards = X16.reshape(N_CORES, SHARD, COLS)
    in_maps = [{"X": shards[i]} for i in range(N_CORES)]

    trace = os.environ.get("BASS_KERNEL_TRACE") == "1"
    res = run_bass_kernel_spmd(nc, in_maps, list(range(N_CORES)), trace=trace)
    if trace:
        LAST_PROFILE.update(
            exec_time_ns=res.exec_time_ns,
            mean_exec_time_ns=res.mean_exec_time_ns,
            trace=res.instructions_and_trace[1] if res.instructions_and_trace else None,
            profile_json=res.profile_json,
        )
    out16 = np.concatenate([res.results[i]["out"] for i in range(N_CORES)], axis=0)
    return out16.astype(np.float32)


# revision 16
# speedup vs baseline: 1.1773x; 1.0586x over previous
"""EquiNN kernel for Trainium2 (Bass, raw), 8-core data parallel, fp16 I/O.

Computes out = l*X + g*rowsum(X) + b for X [4096, 8192] f32.

The rel-err gate is 2e-2 against max|out| (~43), an abs budget of ~0.87.
Casting X to fp16 on the host and streaming fp16 both ways costs ~5e-4
rel err (vs the f64 reference) and halves the per-core DMA bytes:
8.4 MB in + 8.4 MB out. l/g/b are baked as immediates.

Measured on this part: per-core DMA tops out around 430 GB/s on SWDGE
(16 SDMA engines per DMA) and the 8 cores contend for chip HBM, so the
kernel is DMA-bound end to end; compute (~24us across DVE+ACT) hides
under the transfer window. HWDGE-ring stores measured slower in every
layout tried, so all traffic runs on the single SWDGE queue: its FIFO
naturally phases loads before stores with no fabric bubbles.

Per core (shard [512, 8192] = 4 tiles of [128, 8192]):
  - gpsimd: 8 half-tile loads, then per-tile stores gated on affine
    sems, then the drain.
  - DVE: rowsum [0:rd) per tile (tensor_reduce, 1x - a [P,1] dst never
    packs), then sb = g*rs_d + rs_a' in ONE scalar_tensor_tensor, then
    the affine x = l*x + sb as two half-tile tensor_scalars (fp16 4x).
    Same-engine RAW chains between small ops are guarded with self-sem
    waits: raw Bass DVE has no hazard interlock, and the per-partition
    scalar AP of tensor_scalar is prefetched at dispatch.
  - ACT: rowsum [rd:COLS) per tile via Identity(scale=g, bias=b/n) with
    accum_out, so the accumulator directly yields g*sum + b = rs_a'.
"""

import os
from dataclasses import dataclass

import numpy as np

import concourse.bass as bass
from concourse import mybir
from concourse.bass_utils import run_bass_kernel_spmd

N_CORES = 8
ROWS, COLS = 4096, 8192
SHARD = ROWS // N_CORES  # 512 rows per core
P = 128                  # SBUF partitions
N_TILES = SHARD // P     # 4
HALF = COLS // 2         # 4096

LAST_PROFILE = {}


@dataclass(frozen=True)
class Cfg:
    rd: int = 2304  # DVE reduces [0:rd) (1x, ~1.1ns/col); ACT accums the
                    # rest (~0.9-1.06ns/col); both land ~6us/tile


DEFAULT_CFG = Cfg()


def _build(l: float, g: float, b: float, cfg: Cfg = DEFAULT_CFG) -> bass.Bass:
    nc = bass.Bass()
    f16 = mybir.dt.float16
    f32 = mybir.dt.float32
    assert 0 < cfg.rd <= HALF

    # pre-register the const AP the ACT bias needs (only 0.0/1.0 ship)
    bias_val = b / (COLS - cfg.rd)
    bias_t = nc.alloc_sbuf_tensor("const-bias", [P, 1], f32)
    nc.gpsimd.memset(bias_t.ap(), bias_val)
    nc.const_aps.aps[(f32, bias_val)] = bias_t.ap()
    # no barrier: the memset is gpsimd's first instruction and retires well
    # before ACT's first bias read (~15us in)

    X = nc.declare_dram_parameter("X", [SHARD, COLS], f16, isOutput=False)
    out = nc.declare_dram_parameter("out", [SHARD, COLS], f16, isOutput=True)
    Xg = X.rearrange("(t p) c -> t p c", p=P)
    outg = out.rearrange("(t p) c -> t p c", p=P)

    import contextlib

    with contextlib.ExitStack() as ctx:
        xt = [
            ctx.enter_context(nc.sbuf_tensor(f"xt{t}", [P, COLS], f16))
            for t in range(N_TILES)
        ]
        dump = ctx.enter_context(nc.sbuf_tensor("dump", [P, COLS - cfg.rd], f16))
        rs_d = [
            ctx.enter_context(nc.sbuf_tensor(f"rsd{t}", [P, 1], f32))
            for t in range(N_TILES)
        ]
        rs_a = [
            ctx.enter_context(nc.sbuf_tensor(f"rsa{t}", [P, 1], f32))
            for t in range(N_TILES)
        ]
        sb = [
            ctx.enter_context(nc.sbuf_tensor(f"sb{t}", [P, 1], f32))
            for t in range(N_TILES)
        ]
        rs_warm = ctx.enter_context(nc.sbuf_tensor("rs_warm", [P, 1], f32))
        load_sems = [
            [ctx.enter_context(nc.semaphore(f"ld{t}_{k}")) for k in (0, 1)]
            for t in range(N_TILES)
        ]
        act_rs_sems = [
            ctx.enter_context(nc.semaphore(f"act_rs{t}")) for t in range(N_TILES)
        ]
        affine_sems = [
            ctx.enter_context(nc.semaphore(f"aff{t}")) for t in range(N_TILES)
        ]
        psem = ctx.enter_context(nc.semaphore("dve_pipe"))
        store_sem = ctx.enter_context(nc.semaphore("store_sem"))
        block = ctx.enter_context(nc.Block())

        # ---- gpsimd: all loads, then all stores, then drain -------------
        def gpsimd_prog(eng):
            for t in range(N_TILES):
                for k in (0, 1):
                    eng.dma_start(
                        xt[t][:, k * HALF:(k + 1) * HALF],
                        Xg[t][:, k * HALF:(k + 1) * HALF],
                    ).then_inc(load_sems[t][k], 16)
            for t in range(N_TILES):
                eng.wait_ge(affine_sems[t], 2)
                for k in (0, 1):
                    eng.dma_start(
                        outg[t][:, k * HALF:(k + 1) * HALF],
                        xt[t][:, k * HALF:(k + 1) * HALF],
                    ).then_inc(store_sem, 16)
            eng.wait_ge(store_sem, 16 * 2 * N_TILES)

        # ---- DVE: partial rowsum, sb, full affine -----------------------
        def dve_prog(vector):
            for t in range(N_TILES):
                vector.wait_ge(load_sems[t][0], 16)
                nc.vector.reduce_sum(
                    rs_d[t][:], xt[t][:, 0:cfg.rd], axis=mybir.AxisListType.X
                ).then_inc(psem, 1)
                vector.wait_ge(psem, 2 * t + 1)  # rs_d committed
                vector.wait_ge(act_rs_sems[t], 1)
                # sb = g*rs_d + rs_a'   (rs_a' = g*act_sum + b, from ACT)
                nc.vector.scalar_tensor_tensor(
                    sb[t][:], rs_d[t][:], g, rs_a[t][:],
                    op0=mybir.AluOpType.mult, op1=mybir.AluOpType.add,
                ).then_inc(psem, 1)
                vector.wait_ge(psem, 2 * t + 2)  # sb committed before the
                # affine dispatch prefetches it
                for k in (0, 1):
                    # h1 residency guaranteed transitively via act_rs_sems
                    nc.vector.tensor_scalar(
                        xt[t][:, k * HALF:(k + 1) * HALF],
                        xt[t][:, k * HALF:(k + 1) * HALF],
                        l, sb[t][:],
                        op0=mybir.AluOpType.mult, op1=mybir.AluOpType.add,
                    ).then_inc(affine_sems[t], 1)

        # ---- ACT: accum-rowsum of [rd:COLS) -----------------------------
        def act_prog(scalar):
            n_act = COLS - cfg.rd
            # dummy 1-col pass hoists ACT_TABLE_LOAD (~1.3us) into load idle
            nc.scalar.activation(
                dump[:, 0:1], dump[:, 0:1],
                mybir.ActivationFunctionType.Identity,
                bias=b / n_act, scale=g,
                accum_out=rs_warm[:],
            )
            for t in range(N_TILES):
                scalar.wait_ge(load_sems[t][1], 16)
                if cfg.rd < HALF:
                    scalar.wait_ge(load_sems[t][0], 16)
                # accum = sum(g*x + b/n) = g*sum(x) + b
                nc.scalar.activation(
                    dump[:], xt[t][:, cfg.rd:COLS],
                    mybir.ActivationFunctionType.Identity,
                    bias=b / n_act, scale=g,
                    accum_out=rs_a[t][:],
                ).then_inc(act_rs_sems[t], 1)

        block.gpsimd(gpsimd_prog)
        block.vector(dve_prog)
        block.scalar(act_prog)

    return nc


def kernel(X: np.ndarray, l: np.ndarray, g: np.ndarray, b: np.ndarray) -> np.ndarray:
    cfg = DEFAULT_CFG
    nc = _build(float(l[0]), float(g[0]), float(b[0]), cfg)

    X16 = np.ascontiguousarray(X).astype(np.float16)
    shards = X16.reshape(N_CORES, SHARD, COLS)
    in_maps = [{"X": shards[i]} for i in range(N_CORES)]

    trace = os.environ.get("BASS_KERNEL_TRACE") == "1"
    res = run_bass_kernel_spmd(nc, in_maps, list(range(N_CORES)), trace=trace)
    if trace:
        LAST_PROFILE.update(
            exec_time_ns=res.exec_time_ns,
            mean_exec_time_ns=res.mean_exec_time_ns,
            trace=res.instructions_and_trace[1] if res.instructions_and_trace else None,
            profile_json=res.profile_json,
        )
    out16 = np.concatenate([res.results[i]["out"] for i in range(N_CORES)], axis=0)
    return out16.astype(np.float32)
